# revision 2
# baseline (speedup 1.0000x reference)
"""Distributed Trainium2 kernel for AssociativeSparseDistributedMemory.get_cliques.

Reference computation (B=128, INPUT=1024, VCAP=32768, K=32, ACAP=4096, K2=32):
  scores  = keys @ value_proj.T              [B, VCAP]
  idx1    = top_k(scores, 32)
  p       = clique_encoder[idx1].sum(1)      (scale+normalize -- skipped: positive
                                              per-row scale never changes a top-k set)
  scores2 = p @ assoc_proj.T                 [B, ACAP]
  idx2    = top_k(scores2, 32)
  out     = assoc_mem_value[idx2].sum(1)     [B, VCAP]

Distribution over 8 cores:
  - value_proj row-sharded (4096 rows/core): each core computes a [128, 4096]
    score chunk, finds its local top-32 VALUES (4 rounds of DVE max8 +
    match_replace8), all-gathers the 8x32 candidates, and every core derives
    the exact global 32nd-largest value t32 per row.  Selection is then the
    value-threshold mask (scores >= t32) -- no index extraction needed anywhere.
  - clique_encoder row-sharded: p_partial = mask_chunk @ E_shard (mask
    transposed on the PE), then AllReduce(p).
  - assoc_proj row-sharded (pre-transposed on host): scores2 chunk [128, 512],
    same local-top32 + allgather + threshold -> mask2 [128,512]; mask2 is
    all-gathered to give the full [128, 4096] selection w2 on every core.
  - assoc_mem_value column-sharded: out_chunk = w2 @ M_shard (dense matmul --
    the table has 4096 rows and there are 4096 selections, so a dense read
    equals gather traffic).  Host concatenates the 8 [128, 4096] chunks.
"""

import numpy as np

B = 128
INPUT = 1024
VCAP = 32768
ACAP = 4096
K = 32
NCORES = 8
VSH = VCAP // NCORES      # 4096 value rows per core
ASH = ACAP // NCORES      # 512 assoc rows per core

_CACHE = {}

NEG = -1e30


def _build():
    import concourse.bass as bass
    import concourse.mybir as mybir
    import concourse.tile as tile
    from concourse import bacc
    from concourse.masks import make_identity

    f32 = mybir.dt.float32
    Alu = mybir.AluOpType

    nc = bacc.Bacc("TRN2", target_bir_lowering=False, debug=False,
                   num_devices=NCORES)

    # ---- kernel I/O ----
    keysT_d = nc.dram_tensor("keysT", [INPUT, B], f32, kind="ExternalInput")
    vpT_d = nc.dram_tensor("vpT", [INPUT, VSH], f32, kind="ExternalInput")
    E_d = nc.dram_tensor("E", [VSH, ACAP], f32, kind="ExternalInput")
    apT_d = nc.dram_tensor("apT", [ACAP, ASH], f32, kind="ExternalInput")
    M_d = nc.dram_tensor("M", [ACAP, VSH], f32, kind="ExternalInput")
    out_d = nc.dram_tensor("out", [B, VSH], f32, kind="ExternalOutput")

    # ---- internal DRAM bounce buffers for collectives ----
    cand1_in = nc.dram_tensor("cand1_in", [B, K], f32)
    cand1_out = nc.dram_tensor("cand1_out", [B * NCORES, K], f32, addr_space="Shared")
    p_in = nc.dram_tensor("p_in", [B, ACAP], f32)
    p_out = nc.dram_tensor("p_out", [B, ACAP], f32, addr_space="Shared")
    cand2_in = nc.dram_tensor("cand2_in", [B, K], f32)
    cand2_out = nc.dram_tensor("cand2_out", [B * NCORES, K], f32, addr_space="Shared")
    m2_in = nc.dram_tensor("m2_in", [B, ASH], f32)
    m2_out = nc.dram_tensor("m2_out", [B * NCORES, ASH], f32, addr_space="Shared")

    RG = [list(range(NCORES))]

    def topk_vals(nc, pool, src, width, cand, scratch, tag):
        """cand[:, 0:32] = top-32 values of src [128, width], descending per
        8-block. src is left intact; scratch is clobbered."""
        for r in range(4):
            s = src if r == 0 else scratch
            nc.vector.max(out=cand[:, r * 8:(r + 1) * 8], in_=s)
            nc.vector.match_replace(
                out=scratch, in_to_replace=cand[:, r * 8:(r + 1) * 8],
                in_values=s, imm_value=NEG)

    with tile.TileContext(nc) as tc:
        with (
            tc.tile_pool(name="const", bufs=1) as constp,
            tc.tile_pool(name="big", bufs=1) as bigp,
            tc.tile_pool(name="small", bufs=1) as smallp,
            tc.tile_pool(name="rhs", bufs=3) as rhsp,
            tc.tile_pool(name="psum", bufs=8, space="PSUM") as psump,
        ):
            # ---- constants ----
            ident = constp.tile([128, 128], f32)
            make_identity(nc, ident[:, :])
            keysT_sb = constp.tile([128, 8, 128], f32)   # k-chunk, B
            for k in range(8):
                nc.sync.dma_start(out=keysT_sb[:, k, :],
                                  in_=keysT_d[k * 128:(k + 1) * 128, :])

            # ---- stage B: scores chunk = keys @ vp_shard.T  -> [128, 4096] ----
            scores = bigp.tile([B, VSH], f32, tag="A")
            psB = [psump.tile([128, 512], f32, tag="pb", name=f"psB{n}") for n in range(8)]
            for k in range(8):
                rhs = rhsp.tile([128, VSH], f32, tag="rhs")
                nc.sync.dma_start(out=rhs[:, :],
                                  in_=vpT_d[k * 128:(k + 1) * 128, :])
                for n in range(8):
                    nc.tensor.matmul(psB[n][:, :], keysT_sb[:, k, :],
                                     rhs[:, n * 512:(n + 1) * 512],
                                     start=(k == 0), stop=(k == 7))
            for n in range(8):
                nc.scalar.copy(scores[:, n * 512:(n + 1) * 512], psB[n][:, :])

            # ---- stage C: local top-32 values ----
            scratch = bigp.tile([B, VSH], f32, tag="B")
            cand1 = smallp.tile([B, K], f32, tag="c1")
            topk_vals(nc, smallp, scores[:, :], VSH, cand1, scratch[:, :], "t1")

            # ---- stage D: allgather candidates, merge to t32 ----
            nc.sync.dma_start(out=cand1_in[:, :], in_=cand1[:, :])
            nc.gpsimd.collective_compute(
                "AllGather", mybir.AluOpType.bypass, replica_groups=RG,
                ins=[cand1_in.ap().opt()], outs=[cand1_out.ap().opt()])
            cands1 = smallp.tile([B, NCORES * K], f32, tag="cs1")
            for r in range(NCORES):
                nc.sync.dma_start(out=cands1[:, r * K:(r + 1) * K],
                                  in_=cand1_out[r * B:(r + 1) * B, :])
            mcand1 = smallp.tile([B, K], f32, tag="mc1")
            mscr1 = smallp.tile([B, NCORES * K], f32, tag="ms1")
            topk_vals(nc, smallp, cands1[:, :], NCORES * K, mcand1, mscr1[:, :], "mt1")

            # ---- stage F: mask = scores >= t32 ----
            mask1 = bigp.tile([B, VSH], f32, tag="B")
            nc.vector.tensor_scalar(
                out=mask1[:, :], in0=scores[:, :],
                scalar1=mcand1[:, K - 1:K], scalar2=None, op0=Alu.is_ge)

            # ---- stage G: w1T tiles (transpose mask) ----
            w1T = bigp.tile([128, 32, 128], f32, tag="C")
            for t in range(32):
                pt = psump.tile([128, 128], f32, tag="pb")
                nc.tensor.transpose(pt[:, :], mask1[:, t * 128:(t + 1) * 128],
                                    ident[:, :])
                nc.scalar.copy(w1T[:, t, :], pt[:, :])

            # ---- stage H: p_partial = mask1 @ E_shard  [128, 4096] ----
            p_sb = bigp.tile([B, ACAP], f32, tag="A")
            psH = [psump.tile([128, 512], f32, tag="pb", name=f"psH{n}") for n in range(8)]
            for k in range(32):
                rhs = rhsp.tile([128, ACAP], f32, tag="rhs")
                nc.sync.dma_start(out=rhs[:, :],
                                  in_=E_d[k * 128:(k + 1) * 128, :])
                for n in range(8):
                    nc.tensor.matmul(psH[n][:, :], w1T[:, k, :],
                                     rhs[:, n * 512:(n + 1) * 512],
                                     start=(k == 0), stop=(k == 31))
            for n in range(8):
                nc.scalar.copy(p_sb[:, n * 512:(n + 1) * 512], psH[n][:, :])

            # ---- stage I: AllReduce p ----
            nc.sync.dma_start(out=p_in[:, :], in_=p_sb[:, :])
            nc.gpsimd.collective_compute(
                "AllReduce", mybir.AluOpType.add, replica_groups=RG,
                ins=[p_in.ap().opt()], outs=[p_out.ap().opt()])
            p_full = bigp.tile([B, ACAP], f32, tag="B")
            nc.sync.dma_start(out=p_full[:, :], in_=p_out[:, :])

            # ---- stage J: pT tiles ----
            pT = bigp.tile([128, 32, 128], f32, tag="C")
            for t in range(32):
                pt = psump.tile([128, 128], f32, tag="pb")
                nc.tensor.transpose(pt[:, :], p_full[:, t * 128:(t + 1) * 128],
                                    ident[:, :])
                nc.scalar.copy(pT[:, t, :], pt[:, :])

            # ---- stage K: scores2 chunk = p @ assoc_proj_shard.T  [128, 512] ----
            s2 = smallp.tile([B, ASH], f32, tag="s2")
            psK = psump.tile([128, 512], f32, tag="pb")
            for k in range(32):
                rhs = rhsp.tile([128, ASH], f32, tag="rhs")
                nc.sync.dma_start(out=rhs[:, :],
                                  in_=apT_d[k * 128:(k + 1) * 128, :])
                nc.tensor.matmul(psK[:, :], pT[:, k, :], rhs[:, :],
                                 start=(k == 0), stop=(k == 31))
            nc.scalar.copy(s2[:, :], psK[:, :])

            # ---- stage L/M: local top-32, allgather, merge to t32_2 ----
            scr2 = smallp.tile([B, ASH], f32, tag="scr2")
            cand2 = smallp.tile([B, K], f32, tag="c2")
            topk_vals(nc, smallp, s2[:, :], ASH, cand2, scr2[:, :], "t2")
            nc.sync.dma_start(out=cand2_in[:, :], in_=cand2[:, :])
            nc.gpsimd.collective_compute(
                "AllGather", mybir.AluOpType.bypass, replica_groups=RG,
                ins=[cand2_in.ap().opt()], outs=[cand2_out.ap().opt()])
            cands2 = smallp.tile([B, NCORES * K], f32, tag="cs2")
            for r in range(NCORES):
                nc.sync.dma_start(out=cands2[:, r * K:(r + 1) * K],
                                  in_=cand2_out[r * B:(r + 1) * B, :])
            mcand2 = smallp.tile([B, K], f32, tag="mc2")
            mscr2 = smallp.tile([B, NCORES * K], f32, tag="ms2")
            topk_vals(nc, smallp, cands2[:, :], NCORES * K, mcand2, mscr2[:, :], "mt2")

            # ---- stage N/O: mask2, allgather -> w2 [128, 4096] ----
            mask2 = smallp.tile([B, ASH], f32, tag="m2")
            nc.vector.tensor_scalar(
                out=mask2[:, :], in0=s2[:, :],
                scalar1=mcand2[:, K - 1:K], scalar2=None, op0=Alu.is_ge)
            nc.sync.dma_start(out=m2_in[:, :], in_=mask2[:, :])
            nc.gpsimd.collective_compute(
                "AllGather", mybir.AluOpType.bypass, replica_groups=RG,
                ins=[m2_in.ap().opt()], outs=[m2_out.ap().opt()])
            w2 = bigp.tile([B, ACAP], f32, tag="A")
            for r in range(NCORES):
                nc.sync.dma_start(out=w2[:, r * ASH:(r + 1) * ASH],
                                  in_=m2_out[r * B:(r + 1) * B, :])

            # ---- stage P: w2T tiles ----
            w2T = bigp.tile([128, 32, 128], f32, tag="B")
            for t in range(32):
                pt = psump.tile([128, 128], f32, tag="pb")
                nc.tensor.transpose(pt[:, :], w2[:, t * 128:(t + 1) * 128],
                                    ident[:, :])
                nc.scalar.copy(w2T[:, t, :], pt[:, :])

            # ---- stage Q: out chunk = w2 @ M_shard  [128, 4096] ----
            out_sb = bigp.tile([B, VSH], f32, tag="C")
            psQ = [psump.tile([128, 512], f32, tag="pb", name=f"psQ{n}") for n in range(8)]
            for k in range(32):
                rhs = rhsp.tile([128, VSH], f32, tag="rhs")
                nc.sync.dma_start(out=rhs[:, :],
                                  in_=M_d[k * 128:(k + 1) * 128, :])
                for n in range(8):
                    nc.tensor.matmul(psQ[n][:, :], w2T[:, k, :],
                                     rhs[:, n * 512:(n + 1) * 512],
                                     start=(k == 0), stop=(k == 31))
            for n in range(8):
                nc.scalar.copy(out_sb[:, n * 512:(n + 1) * 512], psQ[n][:, :])
            nc.sync.dma_start(out=out_d[:, :], in_=out_sb[:, :])

    nc.compile()
    return nc


def get_nc():
    if "nc" not in _CACHE:
        _CACHE["nc"] = _build()
    return _CACHE["nc"]


def make_in_maps(keys, value_proj, clique_encoder, assoc_proj, assoc_mem_value):
    keysT = np.ascontiguousarray(keys.T.astype(np.float32))
    in_maps = []
    for m in range(NCORES):
        in_maps.append({
            "keysT": keysT,
            "vpT": np.ascontiguousarray(
                value_proj[m * VSH:(m + 1) * VSH, :].T.astype(np.float32)),
            "E": np.ascontiguousarray(
                clique_encoder[m * VSH:(m + 1) * VSH, :].astype(np.float32)),
            "apT": np.ascontiguousarray(
                assoc_proj[m * ASH:(m + 1) * ASH, :].T.astype(np.float32)),
            "M": np.ascontiguousarray(
                assoc_mem_value[:, m * VSH:(m + 1) * VSH].astype(np.float32)),
        })
    return in_maps


def kernel(keys, value_proj, clique_encoder, assoc_proj, assoc_mem_value,
           **run_kwargs):
    from concourse.bass_utils import run_bass_kernel_spmd

    nc = get_nc()
    in_maps = make_in_maps(keys, value_proj, clique_encoder, assoc_proj,
                           assoc_mem_value)
    res = run_bass_kernel_spmd(nc, in_maps, core_ids=list(range(NCORES)),
                               **run_kwargs)
    out = np.concatenate([np.asarray(res.results[m]["out"])
                          for m in range(NCORES)], axis=1)
    _CACHE["last_result"] = res
    return out
